# revision 7
# baseline (speedup 1.0000x reference)
"""Trainium2 Bass kernel for the AttnBlock problem (attention + groupnorm + swish).

Sharding: 8 cores = 4 batches x 2 sequence-halves. Each core receives its
batch's x [128, 4096] with the core's query-half rotated to the front
(attention is permutation invariant over the key/value axis), computes
q for its 2048 tokens, k/v for all 4096 tokens, S^T = K^T Q chunk-wise with
m (keys) on partitions, exp on ScalarE, PV and softmax-denominator
reductions on TensorE with PSUM accumulation, deferred softmax
normalization after the output projection, then GroupNorm stats with a
[32,2] AllReduce over the core pair and a fused scale/shift + Silu epilogue.
"""

import numpy as np

import concourse.bass as bass
import concourse.tile as tile
from concourse import bacc, mybir
from concourse.bass_utils import run_bass_kernel_spmd

F32 = mybir.dt.float32
BF16 = mybir.dt.bfloat16
AF = mybir.ActivationFunctionType
ALU = mybir.AluOpType

C = 128          # channels
N = 4096         # tokens per batch
NLOC = 2048      # query tokens per core
SEC = 1024       # section width (PSUM budget)
NSEC = NLOC // SEC
NCHUNK = N // 128  # key chunks of 128
GN_M = 4 * N     # elements per group for groupnorm stats
EPS = 1e-5

# v2 switch: accumulate softmax denominators on DVE (bf16) instead of
# one ones-matmul per chunk on TensorE.
USE_DVE_SUMS = False


def _emit_chunk_s(nc, ps_s, k_bf, q_bf, sec, j):
    """S^T chunk j for section sec: [m=128, SEC] = K_chunk^T @ Q_sec."""
    ps = ps_s.tile([128, SEC], F32, tag="psA", name=f"ps_s{sec}_{j}")
    lhsT = k_bf[:, j * 128:(j + 1) * 128]
    for h in range(SEC // 512):
        nc.tensor.matmul(
            ps[:, h * 512:(h + 1) * 512],
            lhsT,
            q_bf[:, sec * SEC + h * 512: sec * SEC + (h + 1) * 512],
            start=True, stop=True,
        )
    return ps


def attn_body(tc, x_ext, wqt_ext, wkt_ext, wvt_ext, wot_ext,
              bq_ext, bk_ext, bout_ext, gamma_ext, beta_ext,
              ind_ext, indT_ext, out_ext):
    nc = tc.nc
    with (
        tc.tile_pool(name="const", bufs=1) as const,
        tc.tile_pool(name="big", bufs=1) as big,
        tc.tile_pool(name="mid", bufs=2) as mid,
        tc.tile_pool(name="small", bufs=1) as small,
        tc.tile_pool(name="ptp", bufs=3) as ptp,
        tc.tile_pool(name="ps_s", bufs=2, space="PSUM") as ps_s,
        tc.tile_pool(name="ps_hz", bufs=1, space="PSUM") as ps_hz,
        tc.tile_pool(name="ps_sm", bufs=1, space="PSUM") as ps_sm,
        tc.tile_pool(name="dram", bufs=1, space="DRAM") as dram,
    ):
        # ---- constants ----
        ones_bf = const.tile([128, 1], BF16)
        nc.vector.memset(ones_bf, 1.0)
        ones_row = const.tile([1, 128], F32)
        nc.vector.memset(ones_row, 1.0)
        eps32 = const.tile([32, 1], F32)
        nc.vector.memset(eps32, EPS)

        def load_w(ext, nm):
            wf = const.tile([128, 128], F32, name=nm + "f")
            nc.sync.dma_start(out=wf, in_=ext[:, :])
            wb = const.tile([128, 128], BF16, name=nm + "b")
            nc.vector.tensor_copy(wb, wf)
            return wb

        wqt_bf = load_w(wqt_ext, "wqt")
        wkt_bf = load_w(wkt_ext, "wkt")
        wvt_bf = load_w(wvt_ext, "wvt")
        wot_bf = load_w(wot_ext, "wot")

        def load_vec(ext, nm, p=128):
            t = const.tile([p, 1], F32, name=nm)
            nc.sync.dma_start(out=t, in_=ext[:, :])
            return t

        bq_sb = load_vec(bq_ext, "bq")
        bk_sb = load_vec(bk_ext, "bk")
        bout_sb = load_vec(bout_ext, "bout")
        gamma_sb = load_vec(gamma_ext, "gamma")
        beta_sb = load_vec(beta_ext, "beta")
        ind_sb = const.tile([128, 32], F32)
        nc.sync.dma_start(out=ind_sb, in_=ind_ext[:, :])
        indT_sb = const.tile([32, 128], F32)
        nc.sync.dma_start(out=indT_sb, in_=indT_ext[:, :])

        # ---- x load + bf16 convert (chunked for overlap) ----
        x_f = big.tile([128, N], F32)
        x_bf = big.tile([128, N], BF16)
        for i in range(4):
            sl = slice(i * 1024, (i + 1) * 1024)
            nc.sync.dma_start(out=x_f[:, sl], in_=x_ext[:, sl])
            nc.vector.tensor_copy(x_bf[:, sl], x_f[:, sl])

        # ---- projections ----
        q_bf = big.tile([128, NLOC], BF16)
        k_bf = big.tile([128, N], BF16)
        v0t_bf = big.tile([128, N], BF16)  # chunk j cols [128j:128j+128] = V^T rows

        # K = WkT^T x (+bk): all 4096 cols
        for i in range(4):
            ps_k = ps_s.tile([128, 1024], F32, tag="psA", name=f"ps_k{i}")
            for h in range(2):
                nc.tensor.matmul(
                    ps_k[:, h * 512:(h + 1) * 512],
                    wkt_bf,
                    x_bf[:, i * 1024 + h * 512: i * 1024 + (h + 1) * 512],
                    start=True, stop=True,
                )
            nc.scalar.activation(
                out=k_bf[:, i * 1024:(i + 1) * 1024], in_=ps_k,
                func=AF.Identity, bias=bk_sb, scale=1.0,
            )

        # Q = WqT^T x[:, :2048] (+bq): first half = this core's queries
        for i in range(2):
            ps_q = ps_s.tile([128, 1024], F32, tag="psA", name=f"ps_q{i}")
            for h in range(2):
                nc.tensor.matmul(
                    ps_q[:, h * 512:(h + 1) * 512],
                    wqt_bf,
                    x_bf[:, i * 1024 + h * 512: i * 1024 + (h + 1) * 512],
                    start=True, stop=True,
                )
            nc.scalar.activation(
                out=q_bf[:, i * 1024:(i + 1) * 1024], in_=ps_q,
                func=AF.Identity, bias=bq_sb, scale=1.0,
            )

        # V0T chunks: V0T[:, 128j:128j+128][p, c] = sum_c' x[c', 128j+p] WvT[c', c]
        for j in range(NCHUNK):
            ps_v = ps_s.tile([128, 1024], F32, tag="psA", name=f"ps_v{j}")
            nc.tensor.matmul(
                ps_v[:, 0:128],
                x_bf[:, j * 128:(j + 1) * 128],
                wvt_bf,
                start=True, stop=True,
            )
            nc.vector.tensor_copy(v0t_bf[:, j * 128:(j + 1) * 128], ps_v[:, 0:128])

        # ---- attention + projection epilogue per section ----
        y_full = big.tile([128, NLOC], F32)

        for sec in range(NSEC):
            psum_h = ps_hz.tile([128, SEC], F32, tag="hz", name=f"ps_h{sec}")
            if not USE_DVE_SUMS:
                psum_sm = ps_sm.tile([1, SEC], F32, tag="sm", name=f"ps_sum{sec}")
            else:
                acc_bf = mid.tile([128, SEC], BF16, tag="acc", name=f"acc{sec}")

            s_tiles = {0: _emit_chunk_s(nc, ps_s, k_bf, q_bf, sec, 0)}
            for j in range(NCHUNK):
                if j + 1 < NCHUNK:
                    s_tiles[j + 1] = _emit_chunk_s(nc, ps_s, k_bf, q_bf, sec, j + 1)
                pt = ptp.tile([128, SEC], BF16, tag="pt", name=f"pt{sec}_{j}")
                nc.scalar.activation(out=pt, in_=s_tiles.pop(j), func=AF.Exp)
                lhsT_v = v0t_bf[:, j * 128:(j + 1) * 128]
                for h in range(SEC // 512):
                    nc.tensor.matmul(
                        psum_h[:, h * 512:(h + 1) * 512],
                        lhsT_v,
                        pt[:, h * 512:(h + 1) * 512],
                        start=(j == 0), stop=(j == NCHUNK - 1),
                    )
                if not USE_DVE_SUMS:
                    for h in range(SEC // 512):
                        nc.tensor.matmul(
                            psum_sm[0:1, h * 512:(h + 1) * 512],
                            ones_bf,
                            pt[:, h * 512:(h + 1) * 512],
                            start=(j == 0), stop=(j == NCHUNK - 1),
                        )
                else:
                    if j == 0:
                        nc.vector.tensor_copy(acc_bf, pt)
                    else:
                        nc.vector.tensor_add(acc_bf, acc_bf, pt)

            if USE_DVE_SUMS:
                psum_sm = ps_sm.tile([1, SEC], F32, tag="sm", name=f"ps_sum{sec}")
                for h in range(SEC // 512):
                    nc.tensor.matmul(
                        psum_sm[0:1, h * 512:(h + 1) * 512],
                        ones_bf,
                        acc_bf[:, h * 512:(h + 1) * 512],
                        start=True, stop=True,
                    )

            # h PSUM -> SBUF bf16
            h_bf = mid.tile([128, SEC], BF16, tag="hbf", name=f"h_bf{sec}")
            nc.scalar.copy(h_bf, psum_h)

            # softmax denominators -> reciprocal -> broadcast to 128 partitions
            sums_sb = small.tile([1, SEC], F32, name=f"sums{sec}")
            nc.vector.tensor_copy(sums_sb, psum_sm)
            recip = small.tile([1, SEC], F32, name=f"recip{sec}")
            nc.vector.reciprocal_approx_fast(out=recip, in_=sums_sb)
            # broadcast recip along partitions via a K=1 matmul with ones
            psum_r = ps_sm.tile([128, SEC], F32, tag="sm", name=f"ps_r{sec}")
            for h in range(SEC // 512):
                nc.tensor.matmul(
                    psum_r[:, h * 512:(h + 1) * 512],
                    ones_row,
                    recip[0:1, h * 512:(h + 1) * 512],
                    start=True, stop=True,
                )
            r_sb = mid.tile([128, SEC], F32, tag="rsb", name=f"r_sb{sec}")
            nc.vector.tensor_copy(r_sb, psum_r)

            # z = WoT^T h
            psum_z = ps_hz.tile([128, SEC], F32, tag="hz", name=f"ps_z{sec}")
            for h in range(SEC // 512):
                nc.tensor.matmul(
                    psum_z[:, h * 512:(h + 1) * 512],
                    wot_bf,
                    h_bf[:, h * 512:(h + 1) * 512],
                    start=True, stop=True,
                )

            # y = z * r + b_out + x_resid
            t1 = mid.tile([128, SEC], F32, tag="t1", name=f"t1_{sec}")
            nc.vector.tensor_mul(t1, psum_z, r_sb)
            t2 = mid.tile([128, SEC], F32, tag="t2", name=f"t2_{sec}")
            nc.vector.tensor_scalar(
                out=t2, in0=t1, scalar1=bout_sb, scalar2=None, op0=ALU.add,
            )
            ysl = y_full[:, sec * SEC:(sec + 1) * SEC]
            nc.vector.tensor_add(ysl, t2, x_f[:, sec * SEC:(sec + 1) * SEC])

        # ---- groupnorm stats ----
        sink = big.tile([128, NLOC], BF16)
        stats = small.tile([128, 2], F32)
        nc.scalar.activation(out=sink, in_=y_full, func=AF.Identity,
                             accum_out=stats[:, 0:1])
        nc.scalar.activation(out=sink, in_=y_full, func=AF.Square,
                             accum_out=stats[:, 1:2])

        psum_g = ps_sm.tile([32, 2], F32, tag="sm")
        nc.tensor.matmul(psum_g, ind_sb, stats, start=True, stop=True)
        g_sb = small.tile([32, 2], F32)
        nc.vector.tensor_copy(g_sb, psum_g)

        cc_in = dram.tile([32, 2], F32)
        cc_out = dram.tile([32, 2], F32)
        nc.sync.dma_start(out=cc_in, in_=g_sb)
        nc.gpsimd.collective_compute(
            "AllReduce", ALU.add,
            replica_groups=[[0, 1], [2, 3], [4, 5], [6, 7]],
            ins=[cc_in.opt()], outs=[cc_out.opt()],
        )
        gs = small.tile([32, 2], F32)
        nc.sync.dma_start(out=gs, in_=cc_out)

        # mean/rstd per group
        mv = small.tile([32, 2], F32)
        nc.vector.tensor_scalar(out=mv, in0=gs, scalar1=1.0 / GN_M, scalar2=None,
                                op0=ALU.mult)
        m2 = small.tile([32, 1], F32)
        nc.vector.tensor_mul(m2, mv[:, 0:1], mv[:, 0:1])
        varr = small.tile([32, 1], F32)
        nc.vector.tensor_sub(varr, mv[:, 1:2], m2)
        stdev = small.tile([32, 1], F32)
        nc.scalar.activation(out=stdev, in_=varr, func=AF.Sqrt, bias=eps32,
                             scale=1.0)
        nc.vector.reciprocal(mv[:, 1:2], stdev)

        # broadcast group stats to channels: mc[c, 0]=mean, mc[c, 1]=rstd
        psum_mc = ps_sm.tile([128, 2], F32, tag="sm")
        nc.tensor.matmul(psum_mc, indT_sb, mv, start=True, stop=True)
        mc = small.tile([128, 2], F32)
        nc.vector.tensor_copy(mc, psum_mc)
        scale_c = small.tile([128, 1], F32)
        nc.vector.tensor_mul(scale_c, mc[:, 1:2], gamma_sb)
        tmp_c = small.tile([128, 1], F32)
        nc.vector.tensor_mul(tmp_c, mc[:, 0:1], scale_c)
        shift_c = small.tile([128, 1], F32)
        nc.vector.tensor_sub(shift_c, beta_sb, tmp_c)

        # ---- final normalize + swish + store ----
        for sec in range(NSEC):
            yn = mid.tile([128, SEC], F32, tag="t1", name=f"yn{sec}")
            nc.vector.tensor_scalar(
                out=yn, in0=y_full[:, sec * SEC:(sec + 1) * SEC],
                scalar1=scale_c, scalar2=shift_c,
                op0=ALU.mult, op1=ALU.add,
            )
            sg = mid.tile([128, SEC], F32, tag="sg", name=f"sg{sec}")
            nc.scalar.activation(out=sg, in_=yn, func=AF.Sigmoid)
            o_f = mid.tile([128, SEC], F32, tag="t2", name=f"of{sec}")
            nc.vector.tensor_mul(o_f, yn, sg)
            nc.sync.dma_start(out=out_ext[:, sec * SEC:(sec + 1) * SEC], in_=o_f)


def build_bass():
    nc = bacc.Bacc("TRN2", target_bir_lowering=False, debug=False, num_devices=8)
    x_ext = nc.declare_dram_parameter("x", [C, N], F32, isOutput=False)
    wqt = nc.declare_dram_parameter("wqt", [C, C], F32, isOutput=False)
    wkt = nc.declare_dram_parameter("wkt", [C, C], F32, isOutput=False)
    wvt = nc.declare_dram_parameter("wvt", [C, C], F32, isOutput=False)
    wot = nc.declare_dram_parameter("wot", [C, C], F32, isOutput=False)
    bq = nc.declare_dram_parameter("bq", [C, 1], F32, isOutput=False)
    bk = nc.declare_dram_parameter("bk", [C, 1], F32, isOutput=False)
    bout = nc.declare_dram_parameter("bout", [C, 1], F32, isOutput=False)
    gamma = nc.declare_dram_parameter("gamma", [C, 1], F32, isOutput=False)
    beta = nc.declare_dram_parameter("beta", [C, 1], F32, isOutput=False)
    ind = nc.declare_dram_parameter("ind", [C, 32], F32, isOutput=False)
    indT = nc.declare_dram_parameter("indT", [32, C], F32, isOutput=False)
    out_ext = nc.declare_dram_parameter("out", [C, NLOC], F32, isOutput=True)

    with tile.TileContext(nc) as tc:
        attn_body(tc, x_ext, wqt, wkt, wvt, wot, bq, bk, bout, gamma, beta,
                  ind, indT, out_ext)
    nc.finalize()
    return nc


_NC_CACHE = None


def _get_nc():
    global _NC_CACHE
    if _NC_CACHE is None:
        _NC_CACHE = build_bass()
    return _NC_CACHE


def make_in_maps(inputs):
    x = np.ascontiguousarray(
        np.asarray(inputs["x"], dtype=np.float32).reshape(4, C, N))
    Wq = np.asarray(inputs["Wq"], np.float32)
    Wk = np.asarray(inputs["Wk"], np.float32)
    Wv = np.asarray(inputs["Wv"], np.float32)
    Wo = np.asarray(inputs["Wo"], np.float32)
    bq = np.asarray(inputs["bq"], np.float32)
    bk = np.asarray(inputs["bk"], np.float32)
    bv = np.asarray(inputs["bv"], np.float32)
    bo = np.asarray(inputs["bo"], np.float32)
    gamma = np.asarray(inputs["gamma"], np.float32)
    beta = np.asarray(inputs["beta"], np.float32)

    b_out = (Wo @ bv + bo).astype(np.float32)
    ind = np.zeros((C, 32), np.float32)
    ind[np.arange(C), np.arange(C) // 4] = 1.0
    indT = np.ascontiguousarray(ind.T)

    shared = dict(
        wqt=np.ascontiguousarray(Wq.T), wkt=np.ascontiguousarray(Wk.T),
        wvt=np.ascontiguousarray(Wv.T), wot=np.ascontiguousarray(Wo.T),
        bq=bq.reshape(C, 1), bk=bk.reshape(C, 1), bout=b_out.reshape(C, 1),
        gamma=gamma.reshape(C, 1), beta=beta.reshape(C, 1),
        ind=ind, indT=indT,
    )
    in_maps = []
    for core in range(8):
        b, half = core // 2, core % 2
        xb = x[b]
        # rotate the core's query half to the front (keys are permutation
        # invariant); residual/out use columns [0:2048]
        xc = np.ascontiguousarray(
            np.concatenate([xb[:, half * NLOC:(half + 1) * NLOC],
                            xb[:, (1 - half) * NLOC:(2 - half) * NLOC]], axis=1))
        in_maps.append(dict(x=xc, **shared))
    return in_maps


def assemble_out(results, like_shape=(4, C, 16, 16, 16)):
    out = np.zeros((4, C, N), np.float32)
    for core in range(8):
        b, half = core // 2, core % 2
        out[b, :, half * NLOC:(half + 1) * NLOC] = results[core]["out"]
    return out.reshape(like_shape)


def run(inputs, trace=False, **kw):
    nc = _get_nc()
    in_maps = make_in_maps(inputs)
    res = run_bass_kernel_spmd(nc, in_maps, core_ids=list(range(8)),
                               trace=trace, **kw)
    out = assemble_out(res.results)
    return out, res


def kernel(**inputs):
    out, _ = run(inputs, trace=False)
    return out


# revision 8
# speedup vs baseline: 1.2901x; 1.2901x over previous
"""Trainium2 Bass kernel for the AttnBlock problem (attention + groupnorm + swish).

Sharding: 8 cores = 4 batches x 2 sequence-halves. Each core receives its
batch's x [128, 4096] with the core's query-half rotated to the front
(attention is permutation invariant over the key/value axis), computes
q for its 2048 tokens, k/v for all 4096 tokens, S^T = K^T Q chunk-wise with
m (keys) on partitions, exp on ScalarE, PV on TensorE with PSUM
accumulation, softmax denominators accumulated on DVE and broadcast via a
ones-matmul, deferred softmax normalization after the output projection,
then GroupNorm stats with a [32,2] AllReduce over the core pair and a fused
scale/shift + sigmoid-swish epilogue.
"""

import numpy as np

import concourse.bass as bass
import concourse.tile as tile
from concourse import bacc, mybir
from concourse.bass_utils import run_bass_kernel_spmd

F32 = mybir.dt.float32
BF16 = mybir.dt.bfloat16
AF = mybir.ActivationFunctionType
ALU = mybir.AluOpType

C = 128          # channels
N = 4096         # tokens per batch
NLOC = 2048      # query tokens per core
SEC = 1024       # section width (PSUM budget)
NSEC = NLOC // SEC
NCHUNK = N // 128  # key chunks of 128
GN_M = 4 * N     # elements per group for groupnorm stats
EPS = 1e-5

WARM_COLLECTIVE = True
PAIR_GROUPS = [[0, 1], [2, 3], [4, 5], [6, 7]]


def _emit_chunk_s(nc, ps_s, k_bf, q_bf, sec, j):
    """S^T chunk j for section sec: [m=128, SEC] = K_chunk^T @ Q_sec."""
    ps = ps_s.tile([128, SEC], F32, tag="psA", name=f"ps_s{sec}_{j}")
    lhsT = k_bf[:, j * 128:(j + 1) * 128]
    for h in range(SEC // 512):
        nc.tensor.matmul(
            ps[:, h * 512:(h + 1) * 512],
            lhsT,
            q_bf[:, sec * SEC + h * 512: sec * SEC + (h + 1) * 512],
            start=True, stop=True,
        )
    return ps


def attn_body(tc, x_ext, wqt_ext, wkt_ext, wvt_ext, wot_ext,
              bq_ext, bk_ext, bout_ext, gamma_ext, beta_ext,
              ind_ext, indT_ext, out_ext):
    nc = tc.nc
    with (
        tc.tile_pool(name="const", bufs=1) as const,
        tc.tile_pool(name="big", bufs=1) as big,
        tc.tile_pool(name="mid", bufs=2) as mid,
        tc.tile_pool(name="small", bufs=1) as small,
        tc.tile_pool(name="ptp", bufs=3) as ptp,
        tc.tile_pool(name="ps_s", bufs=2, space="PSUM") as ps_s,
        tc.tile_pool(name="ps_hz", bufs=1, space="PSUM") as ps_hz,
        tc.tile_pool(name="ps_sm", bufs=1, space="PSUM") as ps_sm,
        tc.tile_pool(name="dram", bufs=1, space="DRAM") as dram,
    ):
        # ---- constants ----
        ones_wide = const.tile([128, 128], BF16)
        nc.vector.memset(ones_wide, 1.0)
        eps32 = const.tile([32, 1], F32)
        nc.vector.memset(eps32, EPS)

        def load_w(ext, nm):
            wf = const.tile([128, 128], F32, name=nm + "f")
            nc.sync.dma_start(out=wf, in_=ext[:, :])
            wb = const.tile([128, 128], BF16, name=nm + "b")
            nc.vector.tensor_copy(wb, wf)
            return wb

        wqt_bf = load_w(wqt_ext, "wqt")
        wkt_bf = load_w(wkt_ext, "wkt")
        wvt_bf = load_w(wvt_ext, "wvt")
        wot_bf = load_w(wot_ext, "wot")

        def load_vec(ext, nm, p=128):
            t = const.tile([p, 1], F32, name=nm)
            nc.sync.dma_start(out=t, in_=ext[:, :])
            return t

        bq_sb = load_vec(bq_ext, "bq")
        bk_sb = load_vec(bk_ext, "bk")
        bout_sb = load_vec(bout_ext, "bout")
        gamma_sb = load_vec(gamma_ext, "gamma")
        beta_sb = load_vec(beta_ext, "beta")
        ind_sb = const.tile([128, 32], F32)
        nc.sync.dma_start(out=ind_sb, in_=ind_ext[:, :])
        indT_sb = const.tile([32, 128], F32)
        nc.sync.dma_start(out=indT_sb, in_=indT_ext[:, :])

        # ---- warm-up collective: absorb CC dispatch/ring latency early ----
        if WARM_COLLECTIVE:
            warm_sb = const.tile([32, 2], F32)
            nc.vector.memset(warm_sb, 0.0)
            warm_in = dram.tile([32, 2], F32)
            warm_out = dram.tile([32, 2], F32)
            nc.sync.dma_start(out=warm_in, in_=warm_sb)
            nc.gpsimd.collective_compute(
                "AllReduce", ALU.add, replica_groups=PAIR_GROUPS,
                ins=[warm_in.opt()], outs=[warm_out.opt()],
            )

        # ---- x load + bf16 convert (chunked for overlap) ----
        x_f = big.tile([128, N], F32)
        x_bf = big.tile([128, N], BF16)
        for i in range(8):
            sl = slice(i * 512, (i + 1) * 512)
            nc.sync.dma_start(out=x_f[:, sl], in_=x_ext[:, sl])
        for i in range(4):
            sl = slice(i * 1024, (i + 1) * 1024)
            nc.vector.tensor_copy(x_bf[:, sl], x_f[:, sl])

        # ---- projections ----
        q_bf = big.tile([128, NLOC], BF16)
        k_bf = big.tile([128, N], BF16)
        v0t_bf = big.tile([128, N], BF16)  # chunk j cols [128j:128j+128] = V^T rows

        # K = WkT^T x (+bk): all 4096 cols
        for i in range(4):
            ps_k = ps_s.tile([128, 1024], F32, tag="psA", name=f"ps_k{i}")
            for h in range(2):
                nc.tensor.matmul(
                    ps_k[:, h * 512:(h + 1) * 512],
                    wkt_bf,
                    x_bf[:, i * 1024 + h * 512: i * 1024 + (h + 1) * 512],
                    start=True, stop=True,
                )
            nc.scalar.activation(
                out=k_bf[:, i * 1024:(i + 1) * 1024], in_=ps_k,
                func=AF.Identity, bias=bk_sb, scale=1.0,
            )

        # Q = WqT^T x[:, :2048] (+bq): first half = this core's queries
        for i in range(2):
            ps_q = ps_s.tile([128, 1024], F32, tag="psA", name=f"ps_q{i}")
            for h in range(2):
                nc.tensor.matmul(
                    ps_q[:, h * 512:(h + 1) * 512],
                    wqt_bf,
                    x_bf[:, i * 1024 + h * 512: i * 1024 + (h + 1) * 512],
                    start=True, stop=True,
                )
            nc.scalar.activation(
                out=q_bf[:, i * 1024:(i + 1) * 1024], in_=ps_q,
                func=AF.Identity, bias=bq_sb, scale=1.0,
            )

        # V0T chunks: V0T[:, 128j:128j+128][p, c] = sum_c' x[c', 128j+p] WvT[c', c]
        for j in range(NCHUNK):
            ps_v = ps_s.tile([128, 1024], F32, tag="psA", name=f"ps_v{j}")
            nc.tensor.matmul(
                ps_v[:, 0:128],
                x_bf[:, j * 128:(j + 1) * 128],
                wvt_bf,
                start=True, stop=True,
            )
            nc.vector.tensor_copy(v0t_bf[:, j * 128:(j + 1) * 128], ps_v[:, 0:128])

        # ---- attention + projection epilogue per section ----
        y_full = big.tile([128, NLOC], F32)
        s1_secs, s2_secs = [], []

        for sec in range(NSEC):
            psum_h = ps_hz.tile([128, SEC], F32, tag="hz", name=f"ps_h{sec}")
            acc_bf = mid.tile([128, SEC], BF16, tag="acc", name=f"acc{sec}")

            s_tiles = {0: _emit_chunk_s(nc, ps_s, k_bf, q_bf, sec, 0)}
            for j in range(NCHUNK):
                if j + 1 < NCHUNK:
                    s_tiles[j + 1] = _emit_chunk_s(nc, ps_s, k_bf, q_bf, sec, j + 1)
                pt = ptp.tile([128, SEC], BF16, tag="pt", name=f"pt{sec}_{j}")
                nc.scalar.activation(out=pt, in_=s_tiles.pop(j), func=AF.Exp)
                lhsT_v = v0t_bf[:, j * 128:(j + 1) * 128]
                for h in range(SEC // 512):
                    nc.tensor.matmul(
                        psum_h[:, h * 512:(h + 1) * 512],
                        lhsT_v,
                        pt[:, h * 512:(h + 1) * 512],
                        start=(j == 0), stop=(j == NCHUNK - 1),
                    )
                if j == 0:
                    nc.vector.tensor_copy(acc_bf, pt)
                else:
                    nc.vector.tensor_add(acc_bf, acc_bf, pt)

            # broadcast column sums to all 128 partitions with a ones-matmul
            psum_r = ps_sm.tile([128, SEC], F32, tag="sm", name=f"ps_r{sec}")
            for h in range(SEC // 512):
                nc.tensor.matmul(
                    psum_r[:, h * 512:(h + 1) * 512],
                    ones_wide,
                    acc_bf[:, h * 512:(h + 1) * 512],
                    start=True, stop=True,
                )
            r_sb = mid.tile([128, SEC], F32, tag="rsb", name=f"r_sb{sec}")
            nc.vector.reciprocal_approx_fast(out=r_sb, in_=psum_r)

            # h PSUM -> SBUF bf16 (split so z matmul can start on first half)
            h_bf = mid.tile([128, SEC], BF16, tag="hbf", name=f"h_bf{sec}")
            nc.scalar.copy(h_bf[:, 0:512], psum_h[:, 0:512])
            nc.scalar.copy(h_bf[:, 512:1024], psum_h[:, 512:1024])

            # z = WoT^T h
            psum_z = ps_hz.tile([128, SEC], F32, tag="hz", name=f"ps_z{sec}")
            for h in range(SEC // 512):
                nc.tensor.matmul(
                    psum_z[:, h * 512:(h + 1) * 512],
                    wot_bf,
                    h_bf[:, h * 512:(h + 1) * 512],
                    start=True, stop=True,
                )

            # y = (z * r + b_out) + x_resid, with row-sums accumulated free
            t1 = mid.tile([128, SEC], F32, tag="t1", name=f"t1_{sec}")
            nc.vector.tensor_mul(t1, psum_z, r_sb)
            s1 = small.tile([128, 1], F32, name=f"s1_{sec}")
            ysl = y_full[:, sec * SEC:(sec + 1) * SEC]
            nc.vector.scalar_tensor_tensor(
                out=ysl, in0=t1, scalar=bout_sb,
                in1=x_f[:, sec * SEC:(sec + 1) * SEC],
                op0=ALU.add, op1=ALU.add, accum_out=s1,
            )
            s1_secs.append(s1)
            # sum of squares per channel for this section
            sink = mid.tile([128, SEC], BF16, tag="sink", name=f"sink{sec}")
            s2 = small.tile([128, 1], F32, name=f"s2_{sec}")
            nc.scalar.activation(out=sink, in_=ysl, func=AF.Square, accum_out=s2)
            s2_secs.append(s2)

        # ---- groupnorm stats ----
        stats = small.tile([128, 2], F32)
        nc.vector.tensor_add(stats[:, 0:1], s1_secs[0], s1_secs[1])
        nc.vector.tensor_add(stats[:, 1:2], s2_secs[0], s2_secs[1])

        psum_g = ps_sm.tile([32, 2], F32, tag="sm")
        nc.tensor.matmul(psum_g, ind_sb, stats, start=True, stop=True)
        g_sb = small.tile([32, 2], F32)
        nc.vector.tensor_copy(g_sb, psum_g)

        cc_in = dram.tile([32, 2], F32)
        cc_out = dram.tile([32, 2], F32)
        nc.sync.dma_start(out=cc_in, in_=g_sb)
        nc.gpsimd.collective_compute(
            "AllReduce", ALU.add,
            replica_groups=PAIR_GROUPS,
            ins=[cc_in.opt()], outs=[cc_out.opt()],
        )
        gs = small.tile([32, 2], F32)
        nc.sync.dma_start(out=gs, in_=cc_out)

        # mean/rstd per group
        mv = small.tile([32, 2], F32)
        nc.vector.tensor_scalar(out=mv, in0=gs, scalar1=1.0 / GN_M, scalar2=None,
                                op0=ALU.mult)
        m2 = small.tile([32, 1], F32)
        nc.vector.tensor_mul(m2, mv[:, 0:1], mv[:, 0:1])
        varr = small.tile([32, 1], F32)
        nc.vector.tensor_sub(varr, mv[:, 1:2], m2)
        stdev = small.tile([32, 1], F32)
        nc.scalar.activation(out=stdev, in_=varr, func=AF.Sqrt, bias=eps32,
                             scale=1.0)
        nc.vector.reciprocal(mv[:, 1:2], stdev)

        # broadcast group stats to channels: mc[c, 0]=mean, mc[c, 1]=rstd
        psum_mc = ps_sm.tile([128, 2], F32, tag="sm")
        nc.tensor.matmul(psum_mc, indT_sb, mv, start=True, stop=True)
        mc = small.tile([128, 2], F32)
        nc.vector.tensor_copy(mc, psum_mc)
        scale_c = small.tile([128, 1], F32)
        nc.vector.tensor_mul(scale_c, mc[:, 1:2], gamma_sb)
        tmp_c = small.tile([128, 1], F32)
        nc.vector.tensor_mul(tmp_c, mc[:, 0:1], scale_c)
        shift_c = small.tile([128, 1], F32)
        nc.vector.tensor_sub(shift_c, beta_sb, tmp_c)

        # ---- final normalize + swish + store ----
        for sec in range(NSEC):
            yn = mid.tile([128, SEC], F32, tag="t1", name=f"yn{sec}")
            nc.vector.tensor_scalar(
                out=yn, in0=y_full[:, sec * SEC:(sec + 1) * SEC],
                scalar1=scale_c, scalar2=shift_c,
                op0=ALU.mult, op1=ALU.add,
            )
            sg = mid.tile([128, SEC], F32, tag="sg", name=f"sg{sec}")
            nc.scalar.activation(out=sg, in_=yn, func=AF.Sigmoid)
            o_f = mid.tile([128, SEC], F32, tag="t2", name=f"of{sec}")
            nc.vector.tensor_mul(o_f, yn, sg)
            nc.sync.dma_start(out=out_ext[:, sec * SEC:(sec + 1) * SEC], in_=o_f)


def build_bass():
    nc = bacc.Bacc("TRN2", target_bir_lowering=False, debug=False, num_devices=8)
    x_ext = nc.declare_dram_parameter("x", [C, N], F32, isOutput=False)
    wqt = nc.declare_dram_parameter("wqt", [C, C], F32, isOutput=False)
    wkt = nc.declare_dram_parameter("wkt", [C, C], F32, isOutput=False)
    wvt = nc.declare_dram_parameter("wvt", [C, C], F32, isOutput=False)
    wot = nc.declare_dram_parameter("wot", [C, C], F32, isOutput=False)
    bq = nc.declare_dram_parameter("bq", [C, 1], F32, isOutput=False)
    bk = nc.declare_dram_parameter("bk", [C, 1], F32, isOutput=False)
    bout = nc.declare_dram_parameter("bout", [C, 1], F32, isOutput=False)
    gamma = nc.declare_dram_parameter("gamma", [C, 1], F32, isOutput=False)
    beta = nc.declare_dram_parameter("beta", [C, 1], F32, isOutput=False)
    ind = nc.declare_dram_parameter("ind", [C, 32], F32, isOutput=False)
    indT = nc.declare_dram_parameter("indT", [32, C], F32, isOutput=False)
    out_ext = nc.declare_dram_parameter("out", [C, NLOC], F32, isOutput=True)

    with tile.TileContext(nc) as tc:
        attn_body(tc, x_ext, wqt, wkt, wvt, wot, bq, bk, bout, gamma, beta,
                  ind, indT, out_ext)
    nc.finalize()
    return nc


_NC_CACHE = None


def _get_nc():
    global _NC_CACHE
    if _NC_CACHE is None:
        _NC_CACHE = build_bass()
    return _NC_CACHE


def make_in_maps(inputs):
    x = np.ascontiguousarray(
        np.asarray(inputs["x"], dtype=np.float32).reshape(4, C, N))
    Wq = np.asarray(inputs["Wq"], np.float32)
    Wk = np.asarray(inputs["Wk"], np.float32)
    Wv = np.asarray(inputs["Wv"], np.float32)
    Wo = np.asarray(inputs["Wo"], np.float32)
    bq = np.asarray(inputs["bq"], np.float32)
    bk = np.asarray(inputs["bk"], np.float32)
    bv = np.asarray(inputs["bv"], np.float32)
    bo = np.asarray(inputs["bo"], np.float32)
    gamma = np.asarray(inputs["gamma"], np.float32)
    beta = np.asarray(inputs["beta"], np.float32)

    b_out = (Wo @ bv + bo).astype(np.float32)
    ind = np.zeros((C, 32), np.float32)
    ind[np.arange(C), np.arange(C) // 4] = 1.0
    indT = np.ascontiguousarray(ind.T)

    shared = dict(
        wqt=np.ascontiguousarray(Wq.T), wkt=np.ascontiguousarray(Wk.T),
        wvt=np.ascontiguousarray(Wv.T), wot=np.ascontiguousarray(Wo.T),
        bq=bq.reshape(C, 1), bk=bk.reshape(C, 1), bout=b_out.reshape(C, 1),
        gamma=gamma.reshape(C, 1), beta=beta.reshape(C, 1),
        ind=ind, indT=indT,
    )
    in_maps = []
    for core in range(8):
        b, half = core // 2, core % 2
        xb = x[b]
        # rotate the core's query half to the front (keys are permutation
        # invariant); residual/out use columns [0:2048]
        xc = np.ascontiguousarray(
            np.concatenate([xb[:, half * NLOC:(half + 1) * NLOC],
                            xb[:, (1 - half) * NLOC:(2 - half) * NLOC]], axis=1))
        in_maps.append(dict(x=xc, **shared))
    return in_maps


def assemble_out(results, like_shape=(4, C, 16, 16, 16)):
    out = np.zeros((4, C, N), np.float32)
    for core in range(8):
        b, half = core // 2, core % 2
        out[b, :, half * NLOC:(half + 1) * NLOC] = results[core]["out"]
    return out.reshape(like_shape)


def run(inputs, trace=False, **kw):
    nc = _get_nc()
    in_maps = make_in_maps(inputs)
    res = run_bass_kernel_spmd(nc, in_maps, core_ids=list(range(8)),
                               trace=trace, **kw)
    out = assemble_out(res.results)
    return out, res


def kernel(**inputs):
    out, _ = run(inputs, trace=False)
    return out
